# revision 9
# baseline (speedup 1.0000x reference)
"""Trainium2 Bass kernel for nn_MultiHeadGraphAttentionLayer_28956669509709.

Mathematical reduction used here (verified numerically, norm rel err ~2e-7):

  The reference computes att = softmax(e, axis=-1) and then uses it only as
  sum(att, axis=-1, keepdims=True), which is identically ~1.0 (a softmax row
  sums to one).  Hence out = hp * 1 = hp, the whole [H,B,N,N] attention
  tensor is dead, and `adj` is unused (explicitly identity/drop_edge).

  With a = clip(res_alpha, 0, 1) and Wc = concat_heads(W)  [F_in, H*D]:
      out_b = a * (h_b @ Wc) + (1 - a) * h_b = h_b @ (a * Wc + (1 - a) * I)

  so the whole problem collapses to one [2048,256] @ [256,256] matmul per
  batch sample.  We shard data-parallel over B=8 across the 8 NeuronCores
  (one sample per core), replicating the fused [256,256] weight
  M = a*Wc + (1-a)*I.

  On-device layout: the TensorEngine contracts along the partition dim for
  both operands, so the activation must be f-major.  We pre-transpose h_b on
  host (cheap) and feed it f-major; the kernel then needs zero on-chip
  transposes and produces the output in its natural layout.

Scheduling constraints (walrus permits only ONE HW sync-wait per
instruction on this build):
  - M is packed INTO the activation tensor (columns [0,256) of each load
    chunk), so every matmul reads both operands from a single SBUF tile and
    thus carries at most one DMA-lane wait.
  - Each pair of token tiles accumulates into its own PSUM bank (8 banks,
    never reused) -> no psum-release waits on the TensorEngine.
  - 4 load DMAs + 4 store DMAs = 8 total, one per HWDGE sem lane -> no
    own-lane ordering waits; each store waits only on the DVE copy.

Host-side input layout per core b (X [256, 2560]):
  X[f, :] = [ M[f, :] | h_b[0:1024, f] | M[f, :] | h_b[1024:2048, f] ]
"""

import numpy as np

import concourse.bass as bass
import concourse.tile as tile
from concourse import bacc, mybir
from concourse.bass_utils import run_bass_kernel_spmd

F32 = mybir.dt.float32

# Problem geometry (fixed by the problem spec).
B, N, F_IN = 8, 2048, 256
H, D = 4, 64
FO = H * D              # 256 output features
N_CORES = 8

P = 128                 # SBUF partitions
KC = F_IN // P          # contraction chunks (2)
NCH = 2                 # load chunks per contraction half
TPC = N // P // NCH     # token tiles per chunk (8)
CHUNK_COLS = FO + TPC * P   # 256 + 1024 = 1280 columns per chunk
NB = N // (2 * P)       # PSUM banks used (8); 2 token tiles per bank

_NC_CACHE = {}


def _build_nc(mm_dtype=F32):
    """One-sample SPMD program: out[2048,256] = h @ M, h fed f-major."""
    nc = bacc.Bacc("TRN2", target_bir_lowering=False, debug=False)
    x_d = nc.dram_tensor("x", [F_IN, NCH * CHUNK_COLS], F32, kind="ExternalInput")
    out_d = nc.dram_tensor("out", [N, FO], F32, kind="ExternalOutput")
    # One [128, 1024] SBUF tile covers 512 output rows:
    #   row n = v*512 + q*128 + p  <->  sbuf[p, q*256 + d]
    out_v = out_d.rearrange("(v q p) d -> v p q d", q=4, p=P)

    with tile.TileContext(nc) as tc:
        with (
            tc.tile_pool(name="acts", bufs=1) as hpool,
            tc.tile_pool(name="outs", bufs=1) as opool,
            tc.tile_pool(name="psum", bufs=1, space="PSUM") as pspool,
        ):
            x_tiles = [[None] * NCH for _ in range(KC)]
            for c in range(NCH):
                for k in range(KC):
                    xt = hpool.tile([P, CHUNK_COLS], mm_dtype, tag=f"x{k}_{c}")
                    nc.sync.dma_start(
                        xt[:],
                        x_d[
                            k * P : (k + 1) * P,
                            c * CHUNK_COLS : (c + 1) * CHUNK_COLS,
                        ],
                    )
                    x_tiles[k][c] = xt

            o_tiles = [None] * (NB // 2)
            for u in range(NB):
                ps = pspool.tile([P, 2 * FO], F32, tag=f"bank{u}")
                for q in range(2):
                    t = 2 * u + q
                    c, i = divmod(t, TPC)
                    for k in range(KC):
                        xt = x_tiles[k][c]
                        nc.tensor.matmul(
                            ps[:, q * FO : (q + 1) * FO],
                            xt[:, FO + i * P : FO + (i + 1) * P],
                            xt[:, 0:FO],
                            start=(k == 0),
                            stop=(k == KC - 1),
                        )
                v, half = divmod(u, 2)
                if half == 0:
                    ot = opool.tile([P, 4 * FO], F32, tag=f"o{v}")
                    o_tiles[v] = ot
                nc.vector.tensor_copy(
                    o_tiles[v][:, half * 2 * FO : (half + 1) * 2 * FO], ps[:]
                )
                if half == 1:
                    nc.sync.dma_start(
                        out_v[v],
                        o_tiles[v].rearrange("p (q d) -> p q d", q=4),
                    )

    nc.compile()
    return nc


def _get_nc():
    if "nc" not in _NC_CACHE:
        _NC_CACHE["nc"] = _build_nc()
    return _NC_CACHE["nc"]


def _pack_inputs(h, W, res_alpha):
    a = float(np.clip(np.float32(res_alpha), 0.0, 1.0))
    # Concat heads: out feature index f_out = head*D + d  -> Wc[f_in, f_out]
    Wc = np.transpose(W, (1, 0, 2)).reshape(F_IN, FO)
    M = (a * Wc + (1.0 - a) * np.eye(F_IN, dtype=np.float32)).astype(np.float32)

    in_maps = []
    for b in range(B):
        hT = h[b].T  # [F_IN, N] view
        x = np.empty((F_IN, NCH * CHUNK_COLS), dtype=np.float32)
        for c in range(NCH):
            base = c * CHUNK_COLS
            x[:, base : base + FO] = M
            x[:, base + FO : base + CHUNK_COLS] = hT[
                :, c * TPC * P : (c + 1) * TPC * P
            ]
        in_maps.append({"x": x})
    return in_maps


def kernel(h, adj, W, res_alpha, **_unused):
    h = np.asarray(h, dtype=np.float32)
    W = np.asarray(W, dtype=np.float32)
    assert h.shape == (B, N, F_IN), h.shape
    assert W.shape == (H, F_IN, D), W.shape

    in_maps = _pack_inputs(h, W, res_alpha)
    res = run_bass_kernel_spmd(_get_nc(), in_maps, list(range(N_CORES)))
    out = np.stack([res.results[b]["out"] for b in range(B)], axis=0)
    return out.astype(np.float32)


# revision 11
# speedup vs baseline: 1.8136x; 1.8136x over previous
"""Trainium2 Bass kernel for nn_MultiHeadGraphAttentionLayer_28956669509709.

Mathematical reduction used here (verified numerically, norm rel err ~2e-7):

  The reference computes att = softmax(e, axis=-1) and then uses it only as
  sum(att, axis=-1, keepdims=True), which is identically ~1.0 (a softmax row
  sums to one).  Hence out = hp * 1 = hp, the whole [H,B,N,N] attention
  tensor is dead, and `adj` is unused (explicitly identity/drop_edge).

  With a = clip(res_alpha, 0, 1) and Wc = concat_heads(W)  [F_in, H*D]:
      out_b = a * (h_b @ Wc) + (1 - a) * h_b = h_b @ (a * Wc + (1 - a) * I)

  so the whole problem collapses to one [2048,256] @ [256,256] matmul per
  batch sample.  We shard data-parallel over B=8 across the 8 NeuronCores
  (one sample per core), replicating the fused [256,256] weight
  M = a*Wc + (1-a)*I.

  On-device layout: the TensorEngine contracts along the partition dim for
  both operands, so the activation must be f-major.  We pre-transpose h_b on
  host (cheap) and feed it f-major; the kernel then needs zero on-chip
  transposes and produces the output in its natural layout.

Scheduling constraints (walrus permits only ONE HW sync-wait per
instruction on this build):
  - M is packed INTO the activation tensor (columns [0,256) of each load
    chunk), so every matmul reads both operands from a single SBUF tile and
    thus carries at most one DMA-lane wait.
  - Each pair of token tiles accumulates into its own PSUM bank (8 banks,
    never reused) -> no psum-release waits on the TensorEngine.
  - 4 load DMAs + 4 store DMAs = 8 total, one per HWDGE sem lane -> no
    own-lane ordering waits; each store waits only on the DVE copy.

Host-side input layout per core b (X [256, 2560]):
  X[f, :] = [ M[f, :] | h_b[0:1024, f] | M[f, :] | h_b[1024:2048, f] ]
"""

import numpy as np

import concourse.bass as bass
import concourse.tile as tile
from concourse import bacc, mybir
from concourse.bass_utils import run_bass_kernel_spmd

F32 = mybir.dt.float32

# Problem geometry (fixed by the problem spec).
B, N, F_IN = 8, 2048, 256
H, D = 4, 64
FO = H * D              # 256 output features
N_CORES = 8

P = 128                 # SBUF partitions
KC = F_IN // P          # contraction chunks (2)
NCH = 2                 # load chunks per contraction half
TPC = N // P // NCH     # token tiles per chunk (8)
CHUNK_COLS = FO + TPC * P   # 256 + 1024 = 1280 columns per chunk
NB = N // (2 * P)       # PSUM banks used (8); 2 token tiles per bank

_NC_CACHE = {}


def _build_nc(mm_dtype=F32):
    """One-sample SPMD program: out[2048,256] = h @ M, h fed f-major."""
    nc = bacc.Bacc("TRN2", target_bir_lowering=False, debug=False)
    x_d = nc.dram_tensor(
        "x", [F_IN, NCH * CHUNK_COLS], mm_dtype, kind="ExternalInput"
    )
    out_d = nc.dram_tensor("out", [N, FO], F32, kind="ExternalOutput")
    # One [128, 1024] SBUF tile covers 512 output rows:
    #   row n = v*512 + q*128 + p  <->  sbuf[p, q*256 + d]
    out_v = out_d.rearrange("(v q p) d -> v p q d", q=4, p=P)

    with tile.TileContext(nc) as tc:
        with (
            tc.tile_pool(name="acts", bufs=1) as hpool,
            tc.tile_pool(name="outs", bufs=1) as opool,
            tc.tile_pool(name="psum", bufs=1, space="PSUM") as pspool,
        ):
            x_tiles = [[None] * NCH for _ in range(KC)]
            for c in range(NCH):
                for k in range(KC):
                    xt = hpool.tile([P, CHUNK_COLS], mm_dtype, tag=f"x{k}_{c}")
                    nc.sync.dma_start(
                        xt[:],
                        x_d[
                            k * P : (k + 1) * P,
                            c * CHUNK_COLS : (c + 1) * CHUNK_COLS,
                        ],
                    )
                    x_tiles[k][c] = xt

            o_tiles = [None] * (NB // 2)
            for u in range(NB):
                ps = pspool.tile([P, 2 * FO], F32, tag=f"bank{u}")
                for q in range(2):
                    t = 2 * u + q
                    c, i = divmod(t, TPC)
                    for k in range(KC):
                        xt = x_tiles[k][c]
                        nc.tensor.matmul(
                            ps[:, q * FO : (q + 1) * FO],
                            xt[:, FO + i * P : FO + (i + 1) * P],
                            xt[:, 0:FO],
                            start=(k == 0),
                            stop=(k == KC - 1),
                        )
                v, half = divmod(u, 2)
                if half == 0:
                    ot = opool.tile([P, 4 * FO], F32, tag=f"o{v}")
                    o_tiles[v] = ot
                nc.vector.tensor_copy(
                    o_tiles[v][:, half * 2 * FO : (half + 1) * 2 * FO], ps[:]
                )
                if half == 1:
                    nc.sync.dma_start(
                        out_v[v],
                        o_tiles[v].rearrange("p (q d) -> p q d", q=4),
                    )

    nc.compile()
    return nc


def _get_nc():
    if "nc" not in _NC_CACHE:
        _NC_CACHE["nc"] = _build_nc(mm_dtype=mybir.dt.float32r)
    return _NC_CACHE["nc"]


def _pack_inputs(h, W, res_alpha):
    a = float(np.clip(np.float32(res_alpha), 0.0, 1.0))
    # Concat heads: out feature index f_out = head*D + d  -> Wc[f_in, f_out]
    Wc = np.transpose(W, (1, 0, 2)).reshape(F_IN, FO)
    M = (a * Wc + (1.0 - a) * np.eye(F_IN, dtype=np.float32)).astype(np.float32)

    in_maps = []
    for b in range(B):
        hT = h[b].T  # [F_IN, N] view
        x = np.empty((F_IN, NCH * CHUNK_COLS), dtype=np.float32)
        for c in range(NCH):
            base = c * CHUNK_COLS
            x[:, base : base + FO] = M
            x[:, base + FO : base + CHUNK_COLS] = hT[
                :, c * TPC * P : (c + 1) * TPC * P
            ]
        in_maps.append({"x": x})
    return in_maps


def kernel(h, adj, W, res_alpha, **_unused):
    h = np.asarray(h, dtype=np.float32)
    W = np.asarray(W, dtype=np.float32)
    assert h.shape == (B, N, F_IN), h.shape
    assert W.shape == (H, F_IN, D), W.shape

    in_maps = _pack_inputs(h, W, res_alpha)
    res = run_bass_kernel_spmd(_get_nc(), in_maps, list(range(N_CORES)))
    out = np.stack([res.results[b]["out"] for b in range(B)], axis=0)
    return out.astype(np.float32)
